# revision 6
# baseline (speedup 1.0000x reference)
"""Trainium2 Bass kernel for sparse-in -> dense-hidden -> sampled-out net.

  val1 = relu(in_values @ W1.T[active_in_indices] + b1)        # [B, H]
  val2 = einsum('bh,bkh->bk', val1, W2[active_label_indices]) + b2[...]

Strategy (per core, data-parallel over batch, 16 samples/core):

stage 1: the 2048 (sample, nnz) W1T-row gathers are bucket-sorted into 5
  x 32768-row buckets and fetched with 5 dma_gather instructions (fp32,
  slot-major layout).  The per-slot scaling by in_values AND the
  per-sample segmented reduction are folded into PE accumulation
  matmuls with a host-built [128, 16] one-hot-scale lhsT per slot block:
     val1[s,h] += sum_p SCL[p, s] * G1[p, h]
  -> relu(+b1) -> cast bf16 -> PE transpose -> V1T [128h, 16s].

stage 2: the 65536 (sample, k) W2-row gathers are bucket-sorted into 21
  x 32768-row buckets and fetched with 21 dma_gather instructions
  (bf16, transpose=True) landing PE-ready as Gt [128h, ncols].  One
  matmul per column chunk computes dots against ALL 16 samples
  (out [16, cols]); the host selects each slot's true sample row and
  adds b2 (device never needs per-slot sample routing).

dma_gather notes (HW-verified):
  - int16 indices, [16, n/16] block (idx i at [i%16, i//16]) REPLICATED
    to all 8 GPSIMD-core partition groups -> [128, n/16] tile.
  - transpose=True needs dtype <= 2B; lands rows as columns [h, slot].
  - all pad slots use index 0 (valid row; fetched + discarded on host).
"""

import numpy as np
import ml_dtypes

B, NNZ, F_DIM, H, C, KOUT = 128, 128, 135909, 128, 670091, 4096
N_CORES = 8
BPC = B // N_CORES          # samples per core
BUCKET = 32768              # int16-addressable rows per dma_gather
NB1 = (F_DIM + BUCKET - 1) // BUCKET   # 5 stage-1 buckets
NB2 = (C + BUCKET - 1) // BUCKET       # 21 stage-2 buckets
CHUNK = 1024                # stage-2 matmul/psum column chunk
SUB = 768                   # max num_idxs per dma_gather (>896 crashes ucode)
NQ = 2                      # SWDGE queues (3+ corrupts under 8-core concurrency)
SCRATCH = 1 << 16           # SWDGE descriptor ring bytes

_CACHE = {}


def _roundup(x, m):
    return (x + m - 1) // m * m


def _wrap16(ilist):
    """int16 index list -> [128, n/16] tile (wrapped, replicated x8)."""
    n = len(ilist)
    blk = np.zeros((16, n // 16), np.int16)
    blk[np.arange(n) % 16, np.arange(n) // 16] = ilist
    return np.ascontiguousarray(np.tile(blk, (8, 1)))


def build_program(nblk1, ncols2, h=H, bpc=BPC):
    """nblk1: [NB1] stage-1 slot blocks per bucket; ncols2: [NB2] stage-2
    padded column counts per bucket (multiples of 128)."""
    import concourse.bass as bass
    import concourse.bacc as bacc
    import concourse.mybir as mybir
    import concourse.tile as tile

    fp32 = mybir.dt.float32
    bf16 = mybir.dt.bfloat16
    i16 = mybir.dt.int16

    n1 = int(sum(nblk1)) * 128          # total stage-1 slots
    nc2 = int(sum(ncols2))              # total stage-2 slots
    row1 = [min(BUCKET, F_DIM - b * BUCKET) for b in range(NB1)]
    row2 = [min(BUCKET, C - b * BUCKET) for b in range(NB2)]

    nc = bacc.Bacc(
        "TRN2",
        target_bir_lowering=False,
        debug=False,
        dynamic_dma_scratch_size=SCRATCH,
        num_swdge_queues=NQ,
    )

    w1t = nc.dram_tensor("w1t", [F_DIM, h], fp32, kind="ExternalInput")
    w2b = nc.dram_tensor("w2b", [C, h], bf16, kind="ExternalInput")
    idx1 = nc.dram_tensor("idx1", [128, n1 // 16], i16, kind="ExternalInput")
    idx2 = nc.dram_tensor("idx2", [128, nc2 // 16], i16, kind="ExternalInput")
    scl = nc.dram_tensor("scl", [128, (n1 // 128) * bpc], fp32, kind="ExternalInput")
    b1rep = nc.dram_tensor("b1rep", [bpc, h], fp32, kind="ExternalInput")
    eye = nc.dram_tensor("eye", [bpc, bpc], bf16, kind="ExternalInput")
    out = nc.dram_tensor("val2", [bpc, nc2], fp32, kind="ExternalOutput")

    with tile.TileContext(nc) as tc:
        with (
            tc.tile_pool(name="const", bufs=1) as cpool,
            tc.tile_pool(name="g2", bufs=3) as g2pool,
            tc.tile_pool(name="ob", bufs=4) as opool,
            tc.tile_pool(name="psA", bufs=1, space="PSUM") as psA,
            tc.tile_pool(name="psB", bufs=2, space="PSUM") as psB,
        ):
            ones_col = cpool.tile([128, 1], fp32)
            nc.gpsimd.memset(ones_col[:], 1.0)
            warm_ps = psA.tile([1, 1], fp32, tag="warm")
            nc.tensor.matmul(
                warm_ps[:], lhsT=ones_col[:], rhs=ones_col[:], start=True, stop=True
            )

            # small input loads on the SP HWDGE queue (Pool queue stays
            # free for the gathers)
            idx1_t = cpool.tile([128, n1 // 16], i16)
            nc.sync.dma_start(out=idx1_t[:], in_=idx1[:, :])
            idx2_t = cpool.tile([128, nc2 // 16], i16)
            nc.sync.dma_start(out=idx2_t[:], in_=idx2[:, :])
            scl_t = cpool.tile([128, (n1 // 128) * bpc], fp32)
            nc.sync.dma_start(out=scl_t[:], in_=scl[:, :])
            b1_t = cpool.tile([bpc, h], fp32)
            nc.sync.dma_start(out=b1_t[:], in_=b1rep[:, :])
            eye_t = cpool.tile([bpc, bpc], bf16)
            nc.sync.dma_start(out=eye_t[:], in_=eye[:, :])

            # ---- stage 1 gathers: 5 buckets, fp32, slot-major ----
            qq = [0]

            def next_q():
                q = qq[0]
                qq[0] = (q + 1) % NQ
                return q

            g1 = cpool.tile([128, (n1 // 128) * h], fp32)
            off = 0
            for b in range(NB1):
                nb = int(nblk1[b])
                if nb == 0:
                    continue
                for s0 in range(0, nb, SUB // 128):
                    sb = min(SUB // 128, nb - s0)
                    o = off + s0
                    nc.gpsimd.dma_gather(
                        out_ap=g1[:, o * h : (o + sb) * h].rearrange(
                            "p (b h) -> p b h", b=sb
                        ),
                        in_ap=w1t[b * BUCKET : b * BUCKET + row1[b], :],
                        idxs_ap=idx1_t[:, o * 8 : (o + sb) * 8],
                        num_idxs=sb * 128,
                        num_idxs_reg=sb * 128,
                        elem_size=h,
                        queue_num=next_q(),
                    )
                off += nb

            # ---- stage 2 gathers: 21 buckets, bf16, transposed ----
            g2tiles = []
            boff = 0
            for b in range(NB2):
                ncol = int(ncols2[b])
                g2t = g2pool.tile([128, ncol], bf16, tag="g2")
                for s0 in range(0, ncol, SUB):
                    sw = min(SUB, ncol - s0)
                    nc.gpsimd.dma_gather(
                        out_ap=g2t[:, s0 : s0 + sw].rearrange(
                            "p (o n) -> p o n", o=1
                        ),
                        in_ap=w2b[b * BUCKET : b * BUCKET + row2[b], :],
                        idxs_ap=idx2_t[:, (boff + s0) // 16 : (boff + s0 + sw) // 16],
                        num_idxs=sw,
                        num_idxs_reg=sw,
                        elem_size=h,
                        transpose=True,
                        queue_num=next_q(),
                    )
                g2tiles.append(g2t)
                boff += ncol

            # ---- stage 1 compute: PE accumulation over slot blocks ----
            v1_ps = psA.tile([bpc, h], fp32, tag="v1")
            nblk_tot = n1 // 128
            for q in range(nblk_tot):
                nc.tensor.matmul(
                    v1_ps[:],
                    lhsT=scl_t[:, q * bpc : (q + 1) * bpc],
                    rhs=g1[:, q * h : (q + 1) * h],
                    start=(q == 0),
                    stop=(q == nblk_tot - 1),
                )
            v1f = cpool.tile([bpc, h], fp32)
            import concourse.mybir as mybir2
            nc.vector.tensor_tensor(
                out=v1f[:], in0=v1_ps[:], in1=b1_t[:], op=mybir.AluOpType.add
            )
            v1r = cpool.tile([bpc, h], fp32)
            nc.vector.tensor_scalar_max(v1r[:], v1f[:], 0.0)
            v1b = cpool.tile([bpc, h], bf16)
            nc.vector.tensor_copy(out=v1b[:], in_=v1r[:])
            # transpose -> V1T [h, s] bf16
            v1t_ps = psA.tile([h, bpc], fp32, tag="tr")
            nc.tensor.matmul(
                v1t_ps[:], lhsT=v1b[:], rhs=eye_t[:], start=True, stop=True
            )
            v1t = cpool.tile([h, bpc], bf16)
            nc.scalar.copy(out=v1t[:], in_=v1t_ps[:])

            # ---- stage 2 compute: chunked all-sample dots ----
            # matmul spans must stay inside one 2KB PSUM bank: 512 fp32
            # cols per matmul; pair two matmuls per [16, 1024] psum tile
            # so copies/DMAs run at 1024-col granularity.
            MM = 512
            coff = 0
            ci = 0
            for b in range(NB2):
                ncol = int(ncols2[b])
                g2t = g2tiles[b]
                for c0 in range(0, ncol, CHUNK):
                    cw = min(CHUNK, ncol - c0)
                    ps = psB.tile([bpc, CHUNK], fp32, tag="chunk")
                    for m0 in range(0, cw, MM):
                        mw = min(MM, cw - m0)
                        nc.tensor.matmul(
                            ps[:, m0 : m0 + mw],
                            lhsT=v1t[:],
                            rhs=g2t[:, c0 + m0 : c0 + m0 + mw],
                            start=True,
                            stop=True,
                        )
                    ob = opool.tile([bpc, CHUNK], fp32, tag="ob")
                    if ci % 2 == 0:
                        nc.vector.tensor_copy(out=ob[:, :cw], in_=ps[:, :cw])
                    else:
                        nc.scalar.copy(out=ob[:, :cw], in_=ps[:, :cw])
                    nc.sync.dma_start(
                        out=out[:, coff + c0 : coff + c0 + cw], in_=ob[:, :cw]
                    )
                    ci += 1
                coff += ncol
    nc.finalize()
    return nc


def make_core_inputs(in_values, active_in_indices, active_label_indices,
                     W1T, W2B, b1):
    """Host-side sharding, bucket sort, index/scale layout.

    Returns (in_maps, posts, nblk1, ncols2) where posts[cid] =
    (col_of_pair [bpc, KOUT]) mapping each (sample, k) to its device
    output column.
    """
    bpc, h = BPC, H
    in_maps, posts = [], []
    # shapes must be uniform across cores for SPMD: compute global maxima
    nblk1_g = np.zeros(NB1, np.int64)
    ncols2_g = np.zeros(NB2, np.int64)
    percore = []
    for cid in range(N_CORES):
        s = slice(cid * bpc, (cid + 1) * bpc)
        aii = active_in_indices[s]                  # [bpc, NNZ]
        ali = active_label_indices[s]               # [bpc, KOUT]
        # ---- stage 1: flatten (sample, i), sort by bucket ----
        b1v = (aii // BUCKET).ravel()
        order1 = np.argsort(b1v, kind="stable")
        cnt1 = np.bincount(b1v, minlength=NB1)
        # ---- stage 2 ----
        b2v = (ali // BUCKET).ravel()
        order2 = np.argsort(b2v, kind="stable")
        cnt2 = np.bincount(b2v, minlength=NB2)
        percore.append((order1, cnt1, order2, cnt2))
        nblk1_g = np.maximum(nblk1_g, (cnt1 + 127) // 128)
        ncols2_g = np.maximum(ncols2_g, _roundup(cnt2, 128))
    nblk1 = nblk1_g.astype(int)
    ncols2 = ncols2_g.astype(int)
    n1 = int(nblk1.sum()) * 128
    nc2 = int(ncols2.sum())

    b1rep = np.ascontiguousarray(
        np.broadcast_to(b1.reshape(1, h), (bpc, h))
    ).astype(np.float32)
    eye = np.eye(bpc, dtype=ml_dtypes.bfloat16)

    for cid in range(N_CORES):
        s = slice(cid * bpc, (cid + 1) * bpc)
        aii = active_in_indices[s]
        ali = active_label_indices[s]
        inv = in_values[s]
        order1, cnt1, order2, cnt2 = percore[cid]

        # stage-1 index list + scale blocks
        ilist1 = np.zeros(n1, np.int16)
        scl = np.zeros((128, (n1 // 128) * bpc), np.float32)
        flat_i1 = aii.ravel()[order1]      # bucket-sorted global rows
        flat_s1 = (np.arange(bpc * NNZ) // NNZ)[order1]
        flat_v1 = inv.ravel()[order1]
        pos = 0
        src = 0
        for b in range(NB1):
            cb = int(cnt1[b])
            loc = flat_i1[src : src + cb] - b * BUCKET
            ilist1[pos : pos + cb] = loc.astype(np.int16)
            slots = pos + np.arange(cb)
            scl[slots % 128, (slots // 128) * bpc + flat_s1[src : src + cb]] = (
                flat_v1[src : src + cb]
            )
            pos += int(nblk1[b]) * 128
            src += cb

        # stage-2 index list + output-column mapping
        ilist2 = np.zeros(nc2, np.int16)
        col_of_pair = np.zeros(bpc * KOUT, np.int64)
        flat_i2 = ali.ravel()[order2]
        pos = 0
        src = 0
        for b in range(NB2):
            cb = int(cnt2[b])
            loc = flat_i2[src : src + cb] - b * BUCKET
            ilist2[pos : pos + cb] = loc.astype(np.int16)
            col_of_pair[order2[src : src + cb]] = pos + np.arange(cb)
            pos += int(ncols2[b])
            src += cb

        in_maps.append(
            {
                "w1t": W1T,
                "w2b": W2B,
                "idx1": _wrap16(ilist1),
                "idx2": _wrap16(ilist2),
                "scl": scl,
                "b1rep": b1rep,
                "eye": eye,
            }
        )
        posts.append(col_of_pair.reshape(bpc, KOUT))
    return in_maps, posts, nblk1, ncols2


def postprocess(raw, posts, active_label_indices, b2):
    """raw: list of [bpc, nc2] per core -> full val2 [B, KOUT]."""
    val2 = np.empty((B, KOUT), np.float32)
    srow = np.arange(BPC)[:, None]
    for cid in range(N_CORES):
        s = slice(cid * BPC, (cid + 1) * BPC)
        val2[s] = raw[cid][srow, posts[cid]] + b2[active_label_indices[s]]
    return val2


def kernel(in_values, active_in_indices, active_label_indices, W1, b1, W2, b2):
    from concourse.bass_utils import run_bass_kernel_spmd

    in_values = np.asarray(in_values, dtype=np.float32)
    active_in_indices = np.asarray(active_in_indices, dtype=np.int32)
    active_label_indices = np.asarray(active_label_indices, dtype=np.int32)
    W1 = np.asarray(W1, dtype=np.float32)
    b1 = np.asarray(b1, dtype=np.float32)
    W2 = np.asarray(W2, dtype=np.float32)
    b2 = np.asarray(b2, dtype=np.float32)

    W1T = np.ascontiguousarray(W1.T)
    W2B = np.asarray(W2, dtype=ml_dtypes.bfloat16)
    in_maps, posts, nblk1, ncols2 = make_core_inputs(
        in_values, active_in_indices, active_label_indices, W1T, W2B, b1
    )
    key = (tuple(nblk1), tuple(ncols2))
    if _CACHE.get("key") != key:
        _CACHE["nc"] = build_program(nblk1, ncols2)
        _CACHE["key"] = key
    nc = _CACHE["nc"]

    res = run_bass_kernel_spmd(nc, in_maps, list(range(N_CORES)))
    raw = [r["val2"] for r in res.results]
    val2 = postprocess(raw, posts, active_label_indices, b2)
    return val2, active_label_indices
